# revision 1
# baseline (speedup 1.0000x reference)
"""Trainium2 Bass kernel: NKQuantizer2 top-k masking (k=8).

reference:  kh = topk_hot(x, 8)          # [B,S,Q] 0/1 mask, top-8 per token
            out = einsum('bsq,eq->bse', kh, W)

Per token: out[t] = sum_{q in top8(x[t])} W[:, q] -- an 8-way embedding
gather-sum from W.T [Q, E].

Strategy (data-parallel over tokens across 8 cores, W.T bf16 in HBM):
  Per 128-token tile on each core:
    1. DMA x tile [128, 8192] f32 HBM->SBUF (4-deep prefetch)
    2. DVE Max8 -> top-8 values per token; DVE MaxIndex -> their indices
       (exact, ties -> first occurrence, matching jax.lax.top_k)
    3. 8 single-index indirect DMA gathers with CCE accumulate in the DMA
       datapath: acc[p, :] (+)= WT[idx8[p, j], :]  (bf16 in, f32 out)
    4. DMA acc -> out rows (f32)

Toolchain constraint: at most ONE semaphore wait per instruction. ALL DMAs
ride the single SWDGE FIFO queue (implicit cross-DMA ordering) in an
explicitly pinned pipeline order; buffer pools are sized so every
instruction has cross-proc dependencies on a single other proc.
"""

import numpy as np
import ml_dtypes

import concourse.bass as bass
import concourse.mybir as mybir
import concourse.tile as tile
from concourse.bass_utils import run_bass_kernel_spmd
from concourse.tile_rust import add_dep_helper

B, S, Q, E, TOPK = 4, 2048, 8192, 512, 8
N_CORES = 8
P = 128
T_TOTAL = B * S                 # 8192 tokens
T_CORE = T_TOTAL // N_CORES     # 1024 tokens per core

F32 = mybir.dt.float32
BF16 = mybir.dt.bfloat16
U32 = mybir.dt.uint32


def build_bass(t_core=T_CORE, q=Q, e=E):
    """Build the per-core Bass program (SPMD: same program on all cores)."""
    n_tiles = t_core // P
    xbufs = min(4, n_tiles)

    nc = bass.Bass(trn_type="TRN2", target_bir_lowering=False)
    x_d = nc.dram_tensor("x", [t_core, q], F32, kind="ExternalInput")
    wt_d = nc.dram_tensor("wt", [q, e], BF16, kind="ExternalInput")
    out_d = nc.dram_tensor("out", [t_core, e], F32, kind="ExternalOutput")

    fifo = []  # all SWDGE DMAs in intended FIFO order

    def swdge(dma):
        if fifo:
            add_dep_helper(dma.ins, fifo[-1].ins, False, "fifo order")
        fifo.append(dma)
        return dma

    with tile.TileContext(nc) as tc:
        with (
            tc.tile_pool(name="xpool", bufs=xbufs) as xpool,
            tc.tile_pool(name="spool", bufs=n_tiles) as spool,
            tc.tile_pool(name="ipool", bufs=n_tiles) as ipool,
            tc.tile_pool(name="gpool", bufs=n_tiles) as gpool,
        ):
            xts = [xpool.tile([P, q], F32, name="xt", tag="xt") for _ in range(xbufs)]
            idx8s, g8s, i_idxs, lastadds, ostores = [], [], [], [], []

            def emit_xload(i):
                xt = xts[i % xbufs]
                dma = nc.sync.dma_start(xt[:], x_d[i * P : (i + 1) * P, :])
                if i >= xbufs:
                    add_dep_helper(
                        dma.ins, i_idxs[i - xbufs].ins, True, "xt WAR"
                    )
                    # The WAW edge to the old x-load is redundant: the WAR on
                    # its readers already orders the writes at runtime.
                    dma.ins.try_remove_dependency(xls[i - xbufs].ins.name)
                return dma

            def emit_topk(i):
                xt = xts[i % xbufs]
                s8 = spool.tile([P, 8], F32, name="s8", tag="s8")
                nc.vector.max(out=s8[:], in_=xt[:])
                idx8 = ipool.tile([P, 8], U32, name="idx8", tag="idx8")
                i_idx = nc.vector.max_index(
                    out=idx8[:], in_max=s8[:], in_values=xt[:]
                )
                idx8s.append(idx8)
                i_idxs.append(i_idx)
                g8s.append(gpool.tile([P, e], F32, name="g8", tag="g8"))

            def emit_gather(i, j):
                swdge(
                    nc.gpsimd.indirect_dma_start(
                        out=g8s[i][:],
                        out_offset=None,
                        in_=wt_d[:],
                        in_offset=bass.IndirectOffsetOnAxis(
                            ap=idx8s[i][:, j : j + 1], axis=0
                        ),
                        compute_op=(
                            mybir.AluOpType.bypass
                            if j == 0
                            else mybir.AluOpType.add
                        ),
                    )
                )

            def emit_ostore(i):
                dma = swdge(
                    nc.gpsimd.dma_start(
                        out_d[i * P : (i + 1) * P, :], g8s[i][:]
                    )
                )
                ostores.append(dma)
                return dma

            # x-loads ride the HWDGE ring (8 loads, 8 fresh lanes, one wait
            # each); the SWDGE FIFO carries only gathers + stores, wave-major,
            # so tile i owns SWDGE lane i: its first gather carries the one
            # idx8 wait and every later gather/store's only wait is its
            # same-lane predecessor (= its accumulate-chain dependency).
            xls = []
            for i in range(n_tiles):
                xls.append(emit_xload(i))
                emit_topk(i)
            for j in range(TOPK):
                for i in range(n_tiles):
                    emit_gather(i, j)
            for i in range(n_tiles):
                emit_ostore(i)

            # Quiesce procs with single-wait SP nops so the kernel-tail
            # drains find their required ticks already observed.
            tail = xls + fifo[-10:] + i_idxs[-1:]
            for tgt in tail:
                n = nc.sync.nop()
                add_dep_helper(n.ins, tgt.ins, True, "tail quiesce")

    return nc


def _prep_wt(W: np.ndarray) -> np.ndarray:
    """W [e, q] f32 -> WT [q, e] bf16 contiguous."""
    return np.ascontiguousarray(W.T).astype(ml_dtypes.bfloat16)


_CACHED = {}


def _get_nc():
    if "nc" not in _CACHED:
        _CACHED["nc"] = build_bass()
    return _CACHED["nc"]


def kernel(x: np.ndarray, W: np.ndarray) -> np.ndarray:
    x = np.asarray(x, dtype=np.float32)
    W = np.asarray(W, dtype=np.float32)
    assert x.shape == (B, S, Q) and W.shape == (E, Q)

    nc = _get_nc()
    xf = x.reshape(T_TOTAL, Q)
    WT = _prep_wt(W)
    in_maps = [
        {
            "x": np.ascontiguousarray(xf[c * T_CORE : (c + 1) * T_CORE]),
            "wt": WT,
        }
        for c in range(N_CORES)
    ]
    res = run_bass_kernel_spmd(nc, in_maps, core_ids=list(range(N_CORES)))
    out = np.concatenate([r["out"] for r in res.results], axis=0)
    return np.ascontiguousarray(out.reshape(B, S, E).astype(np.float32))



# revision 11
# speedup vs baseline: 1.2671x; 1.2671x over previous
"""Trainium2 Bass kernel: NKQuantizer2 top-k masking (k=8).

reference:  kh = topk_hot(x, 8)          # [B,S,Q] 0/1 mask, top-8 per token
            out = einsum('bsq,eq->bse', kh, W)

Per token: out[t] = sum_{q in top8(x[t])} W[:, q] -- an 8-way embedding
gather-sum from W.T [Q, E].

Strategy (data-parallel over tokens across 8 cores, W.T bf16 in HBM):
  Per 128-token tile on each core:
    1. DMA x tile [128, 8192] f32 HBM->SBUF (4-deep prefetch)
    2. DVE Max8 -> top-8 values per token; DVE MaxIndex -> their indices
       (exact, ties -> first occurrence, matching jax.lax.top_k)
    3. 8 single-index indirect DMA gathers with CCE accumulate in the DMA
       datapath: acc[p, :] (+)= WT[idx8[p, j], :]  (bf16 in, f32 out)
    4. DMA acc -> out rows (f32)

Toolchain constraint: at most ONE semaphore wait per instruction. ALL DMAs
ride the single SWDGE FIFO queue (implicit cross-DMA ordering) in an
explicitly pinned pipeline order; buffer pools are sized so every
instruction has cross-proc dependencies on a single other proc.
"""

import numpy as np
import ml_dtypes

import concourse.bass as bass
import concourse.mybir as mybir
import concourse.tile as tile
from concourse.bass_utils import run_bass_kernel_spmd
from concourse.tile_rust import add_dep_helper

B, S, Q, E, TOPK = 4, 2048, 8192, 512, 8
N_CORES = 8
P = 128
T_TOTAL = B * S                 # 8192 tokens
T_CORE = T_TOTAL // N_CORES     # 1024 tokens per core

F32 = mybir.dt.float32
BF16 = mybir.dt.bfloat16
U32 = mybir.dt.uint32


def build_bass(t_core=T_CORE, q=Q, e=E):
    """Build the per-core Bass program (SPMD: same program on all cores)."""
    n_tiles = t_core // P
    xbufs = min(4, n_tiles)

    nc = bass.Bass(trn_type="TRN2", target_bir_lowering=False)
    x_d = nc.dram_tensor("x", [t_core, q], F32, kind="ExternalInput")
    wt_d = nc.dram_tensor("wt", [q, e], BF16, kind="ExternalInput")
    out_d = nc.dram_tensor("out", [t_core, e], F32, kind="ExternalOutput")

    fifo = []  # all SWDGE DMAs in intended FIFO order

    def swdge(dma):
        if fifo:
            add_dep_helper(dma.ins, fifo[-1].ins, False, "fifo order")
        fifo.append(dma)
        return dma

    with tile.TileContext(nc) as tc:
        with (
            tc.tile_pool(name="xpool", bufs=xbufs) as xpool,
            tc.tile_pool(name="spool", bufs=n_tiles) as spool,
            tc.tile_pool(name="ipool", bufs=n_tiles) as ipool,
            tc.tile_pool(name="gpool", bufs=n_tiles) as gpool,
        ):
            xts = [xpool.tile([P, q], F32, name="xt", tag="xt") for _ in range(xbufs)]
            idx8s, g8s, i_idxs, lastadds, ostores = [], [], [], [], []

            def emit_xload(i):
                xt = xts[i % xbufs]
                dma = nc.sync.dma_start(xt[:], x_d[i * P : (i + 1) * P, :])
                if i >= xbufs:
                    add_dep_helper(
                        dma.ins, i_idxs[i - xbufs].ins, True, "xt WAR"
                    )
                    # The WAW edge to the old x-load is redundant: the WAR on
                    # its readers already orders the writes at runtime.
                    dma.ins.try_remove_dependency(xls[i - xbufs].ins.name)
                return dma

            def emit_topk(i):
                xt = xts[i % xbufs]
                s8 = spool.tile([P, 8], F32, name="s8", tag="s8")
                nc.vector.max(out=s8[:], in_=xt[:])
                idx8 = ipool.tile([P, 8], U32, name="idx8", tag="idx8")
                i_idx = nc.vector.max_index(
                    out=idx8[:], in_max=s8[:], in_values=xt[:]
                )
                idx8s.append(idx8)
                i_idxs.append(i_idx)
                g8s.append(gpool.tile([P, e], F32, name="g8", tag="g8"))

            def emit_gather(i, j):
                nop = None
                if j == 0 and i > 0:
                    # Hoist the DVE (idx8 ready) wait onto a Pool nop so the
                    # first gather carries only its sem-lane wait (walrus
                    # allows a single wait per DMA instruction).
                    nop = nc.gpsimd.nop()
                    add_dep_helper(nop.ins, i_idxs[i].ins, True, "idx8 ready")
                dma = nc.gpsimd.indirect_dma_start(
                    out=g8s[i][:],
                    out_offset=None,
                    in_=wt_d[:],
                    in_offset=bass.IndirectOffsetOnAxis(
                        ap=idx8s[i][:, j : j + 1], axis=0
                    ),
                    compute_op=(
                        mybir.AluOpType.bypass
                        if j == 0
                        else mybir.AluOpType.add
                    ),
                )
                if j > 0:
                    # The g8 accumulate chain is ordered by the SWDGE ring
                    # itself: one logical queue, per-engine FIFO rings, and a
                    # fixed partition->engine map serialize same-partition CCE
                    # adds. Demote the completion-sem dep (which would stall
                    # Q7 desc-gen on the full DMA latency of gather j-1) to a
                    # nosync (ring-order) edge.
                    dma.ins.try_remove_dependency(fifo[-1].ins.name)
                    # idx8 read is ordered behind gather j=0 (which carries
                    # the DVE wait) via Pool program order + FIFO nosync.
                    dma.ins.try_remove_dependency(i_idxs[i].ins.name)
                    add_dep_helper(dma.ins, i_idxs[i].ins, False, "idx8 via g0")
                if nop is not None:
                    # Pin the nop before the gather on the Pool sequencer so
                    # its DVE wait is observed by the time the gather issues.
                    add_dep_helper(dma.ins, nop.ins, False, "after idx8 nop")
                    dma.ins.try_remove_dependency(i_idxs[i].ins.name)
                swdge(dma)

            def emit_ostore(i):
                # SWDGE FIFO right behind tile i's last gather: ring order
                # covers the g8 read-after-write, so demote that dep too.
                dma = nc.gpsimd.dma_start(
                    out_d[i * P : (i + 1) * P, :], g8s[i][:]
                )
                dma.ins.try_remove_dependency(fifo[-1].ins.name)
                swdge(dma)
                ostores.append(dma)
                return dma

            # x-loads ride the HWDGE SP ring (one wait each); the SWDGE FIFO
            # carries the gathers TILE-MAJOR: tile i's first gather waits on
            # idx8_i (ready while the FIFO is still draining tile i-1), so
            # the Q7 descriptor stream chases the DVE with no head-of-line
            # stall. Stores ride the ACT HWDGE ring.
            xls = []
            for i in range(n_tiles):
                xls.append(emit_xload(i))
                emit_topk(i)
                for j in range(TOPK):
                    emit_gather(i, j)
                emit_ostore(i)

            # Quiesce procs with single-wait SP nops so the kernel-tail
            # drains find their required ticks already observed.
            tail = xls + fifo[-10:] + i_idxs[-1:]
            for tgt in tail:
                n = nc.sync.nop()
                add_dep_helper(n.ins, tgt.ins, True, "tail quiesce")

    return nc


def _prep_wt(W: np.ndarray) -> np.ndarray:
    """W [e, q] f32 -> WT [q, e] bf16 contiguous."""
    return np.ascontiguousarray(W.T).astype(ml_dtypes.bfloat16)


_CACHED = {}


def _get_nc():
    if "nc" not in _CACHED:
        _CACHED["nc"] = build_bass()
    return _CACHED["nc"]


def kernel(x: np.ndarray, W: np.ndarray) -> np.ndarray:
    x = np.asarray(x, dtype=np.float32)
    W = np.asarray(W, dtype=np.float32)
    assert x.shape == (B, S, Q) and W.shape == (E, Q)

    nc = _get_nc()
    xf = x.reshape(T_TOTAL, Q)
    WT = _prep_wt(W)
    in_maps = [
        {
            "x": np.ascontiguousarray(xf[c * T_CORE : (c + 1) * T_CORE]),
            "wt": WT,
        }
        for c in range(N_CORES)
    ]
    res = run_bass_kernel_spmd(nc, in_maps, core_ids=list(range(N_CORES)))
    out = np.concatenate([r["out"] for r in res.results], axis=0)
    return np.ascontiguousarray(out.reshape(B, S, E).astype(np.float32))



# revision 58
# speedup vs baseline: 1.3399x; 1.0575x over previous
"""Trainium2 Bass kernel: NKQuantizer2 top-k masking (k=8).

reference:  kh = topk_hot(x, 8)          # [B,S,Q] 0/1 mask, top-8 per token
            out = einsum('bsq,eq->bse', kh, W)

Per token: out[t] = sum_{q in top8(x[t])} W[:, q] -- an 8-way embedding
gather-sum from W.T [Q, E].

Strategy (data-parallel over tokens across 8 cores; WT bf16 in HBM).
Per 128-token tile on each core:
  1. HWDGE DMA x tile [128, 8192] f32 HBM->SBUF (SP ring, 3-deep)
  2. DVE Max8 -> top-8 values; DVE MaxIndex -> their indices (exact,
     duplicate values resolved to successive positions, matching
     jax.lax.top_k first-occurrence tie-break)
  3. ONE SWDGE multi-index gather: G[p, j, :] = WT[idx8[p, j], :]
     (1024 descriptors of 1 KiB in a single instruction -- one Q7
     descriptor-gen pass, one DVE semaphore wait)
  4. GPSIMD tensor_tensor add tree over the j axis (3 levels, Pool
     engine -- DVE stays free): g8 = sum_j G[:, j, :]
  5. SWDGE store g8 -> out rows

Every DMA instruction may carry at most ONE semaphore wait (walrus
limit). Deps whose source is a Pool-engine instruction are demoted to
nosync ordering edges for SWDGE DMAs: their descriptor generation runs
on the Pool sequencer AFTER that instruction completed (program order),
so the semaphore would be redundant.
"""

import numpy as np
import ml_dtypes

import concourse.bass as bass
import concourse.mybir as mybir
import concourse.tile as tile
from concourse.bass_utils import run_bass_kernel_spmd
from concourse.tile_rust import add_dep_helper

B, S, Q, E, TOPK = 4, 2048, 8192, 512, 8
N_CORES = 8
P = 128
T_TOTAL = B * S                 # 8192 tokens
T_CORE = T_TOTAL // N_CORES     # 1024 tokens per core

F32 = mybir.dt.float32
BF16 = mybir.dt.bfloat16
U32 = mybir.dt.uint32


def build_bass(t_core=T_CORE, q=Q, e=E):
    """Build the per-core Bass program (SPMD: same program on all cores)."""
    n_tiles = t_core // P
    xbufs = min(3, n_tiles)

    nc = bass.Bass(trn_type="TRN2", target_bir_lowering=False)
    x_d = nc.dram_tensor("x", [t_core, q], F32, kind="ExternalInput")
    wt_d = nc.dram_tensor("wt", [q, e], BF16, kind="ExternalInput")
    out_d = nc.dram_tensor("out", [t_core, e], F32, kind="ExternalOutput")

    pool_insts = {}  # name -> Pool-engine compute instruction
    tt3_names = set()

    def demote_pool_deps(inst, keep=()):
        """Demote an instruction's sync deps on Pool COMPUTE instructions to
        nosync edges: the Pool engine executes serially, so anything issued
        later on Pool (compute or SWDGE descriptor-gen) starts only after
        those completed. Deps on DMA instructions (async data landing) are
        kept."""
        keep_names = {k.ins.name for k in keep}
        for name in list(inst.ins.sync_dependency_names()):
            if name in pool_insts and name not in keep_names:
                inst.ins.try_remove_dependency(name)
                add_dep_helper(inst.ins, pool_insts[name].ins, False, "Pool order")
        return inst

    with tile.TileContext(nc) as tc:
        with (
            tc.tile_pool(name="xpool", bufs=xbufs) as xpool,
            tc.tile_pool(name="spool", bufs=n_tiles) as spool,
            tc.tile_pool(name="ipool", bufs=n_tiles) as ipool,
            tc.tile_pool(name="Gpool", bufs=n_tiles) as Gpool,
            tc.tile_pool(name="gpool", bufs=n_tiles) as gpool,
        ):
            xts = [xpool.tile([P, q], F32, name="xt", tag="xt") for _ in range(xbufs)]
            i_idxs = []
            xls = []
            xl_names = set()
            swdge_dmas = []
            swdge_names = {}
            pending = []
            finals = {}
            dve_insts = {}
            dve_latest = [None]

            def reg_dve(op):
                dve_insts[op.ins.name] = op
                dve_latest[0] = op
                return op

            def emit_reduce_store(i, Gq, b_g):
                # Two in-place quarter adds on DVE (ordered after the newest
                # max_index via a nosync edge so they never block top-k),
                # final add on Pool, then the SWDGE store.
                # A nop ladder waits each quarter's own completion sem (one
                # wait per instruction), then one merged add folds quarters
                # 2,3 into 0,1.
                ladder = []
                for bg in b_g:
                    vn = reg_dve(nc.vector.nop())
                    add_dep_helper(vn.ins, bg.ins, True, "quarter done")
                    ladder.append(vn)
                vn1 = ladder[-1]
                va1 = reg_dve(nc.vector.tensor_tensor(
                    out=Gq[:, 0:2, :], in0=Gq[:, 0:2, :], in1=Gq[:, 2:4, :],
                    op=mybir.AluOpType.add,
                ))
                for vn in ladder:
                    # The add must sit AFTER the whole ladder in DVE program
                    # order -- without these edges the scheduler may hoist it
                    # above the waits.
                    add_dep_helper(va1.ins, vn.ins, False, "after ladder")
                    add_dep_helper(vn.ins, i_idxs[-1].ins, False, "after topk")
                for bg in b_g:
                    va1.ins.try_remove_dependency(bg.ins.name)
                    add_dep_helper(va1.ins, bg.ins, False, "via ring nop")
                # Hoist any remaining cross-engine sync deps (scheduler
                # artifacts on tail tiles) onto their own nops; demote
                # same-engine (DVE) deps to program order.
                for name in list(va1.ins.sync_dependency_names()):
                    if name in dve_insts:
                        va1.ins.try_remove_dependency(name)
                        add_dep_helper(va1.ins, dve_insts[name].ins, False, "DVE order")
                    elif name in swdge_names:
                        va1.ins.try_remove_dependency(name)
                        hn = reg_dve(nc.vector.nop())
                        add_dep_helper(hn.ins, swdge_names[name].ins, True, "hoist")
                        add_dep_helper(va1.ins, hn.ins, False, "after hoist nop")
                va2 = va1
                for v in (vn1, va1):
                    # Keep the whole reduce AFTER the newest max_index in DVE
                    # program order -- it waits on the gather round trip and
                    # must never stall the top-k stream.
                    add_dep_helper(v.ins, i_idxs[-1].ins, False, "after topk")
                tt3 = nc.gpsimd.tensor_tensor(
                    out=Gq[:, 0, :], in0=Gq[:, 0, :], in1=Gq[:, 1, :],
                    op=mybir.AluOpType.add,
                )
                pool_insts[tt3.ins.name] = tt3
                tt3_names.add(tt3.ins.name)
                st = nc.gpsimd.dma_start(
                    out_d[i * P : (i + 1) * P, :], Gq[:, 0, :]
                )
                demote_pool_deps(st)
                swdge_dmas.append(st)
                swdge_names[st.ins.name] = st
                finals["tt"] = tt3
                finals["va"] = va2
            for i in range(n_tiles):
                xt = xts[i % xbufs]
                # Load x in 4 column chunks: a single 4 MiB DMA queues 256
                # KiB per SDMA engine, which delays the gathers' completion
                # semaphores by up to ~9 us; 1 MiB chunks cap that at ~2.5 us.
                # All chunks ride the SP HWDGE ring in order, so chunk 3's
                # completion implies chunks 0-2 have landed on every engine.
                chunks = []
                spn = None
                if i >= xbufs:
                    # One SP nop carries every xt-slot WAR: the old slot
                    # readers are max/max_index of tile i-xbufs, covered by
                    # that tile's max_index tick. Tiles 0..xbufs-1 write
                    # fresh buffers -- any DVE dep on their chunks is a
                    # tracking artifact, dropped without replacement.
                    spn = nc.sync.nop()
                    add_dep_helper(spn.ins, i_idxs[i - xbufs].ins, True, "xt WAR")
                for c4 in range(4):
                    lo, hi = c4 * (q // 4), (c4 + 1) * (q // 4)
                    xc = nc.sync.dma_start(
                        xt[:, lo:hi], x_d[i * P : (i + 1) * P, lo:hi]
                    )
                    for name in list(xc.ins.sync_dependency_names()):
                        if name in dve_insts:
                            xc.ins.try_remove_dependency(name)
                        elif name in xl_names:
                            # old-chunk WAW, redundant under the WAR
                            xc.ins.try_remove_dependency(name)
                    if spn is not None:
                        add_dep_helper(xc.ins, spn.ins, False, "after WAR nop")
                    chunks.append(xc)
                xls.extend(chunks)
                for xc in chunks:
                    xl_names.add(xc.ins.name)

                s8 = spool.tile([P, 8], F32, name="s8", tag="s8")
                vm = reg_dve(nc.vector.max(out=s8[:], in_=xt[:]))
                # One wait: the last chunk's completion implies the rest.
                for xc in chunks[:-1]:
                    vm.ins.try_remove_dependency(xc.ins.name)
                    add_dep_helper(vm.ins, xc.ins, False, "ring order")
                idx8 = ipool.tile([P, 8], U32, name="idx8", tag="idx8")
                i_idx = reg_dve(nc.vector.max_index(
                    out=idx8[:], in_max=s8[:], in_values=xt[:]
                ))
                # max8 already waited on the last chunk; DVE order covers it.
                for xc in chunks:
                    i_idx.ins.try_remove_dependency(xc.ins.name)
                    add_dep_helper(i_idx.ins, xc.ins, False, "via max8 wait")
                i_idxs.append(i_idx)

                # 8 single-offset gathers (the HW-supported form) arranged as
                # 4 CCE-accumulate chains of depth 2 into quarters:
                #   Gq[:, j, :]  = WT[idx8[:, j]]        (bypass, bf16->f32)
                #   Gq[:, j, :] += WT[idx8[:, j+4]]      (CCE add)
                # Chain waits are hoisted onto Pool nops so each DMA carries
                # at most one (sem-lane) wait.
                Gq = Gpool.tile([P, 4, e], F32, name="Gq", tag="Gq")
                if i > 0:
                    # Hoist the idx8-ready (DVE) wait for this tile's gathers.
                    n = nc.gpsimd.nop()
                    add_dep_helper(n.ins, i_idx.ins, True, "idx8 ready")
                    pool_insts[n.ins.name] = n

                def gather(j, acc):
                    gd = nc.gpsimd.indirect_dma_start(
                        out=Gq[:, j % 4, :],
                        out_offset=None,
                        in_=wt_d[:],
                        in_offset=bass.IndirectOffsetOnAxis(
                            ap=idx8[:, j : j + 1], axis=0
                        ),
                        compute_op=(
                            mybir.AluOpType.add if acc else mybir.AluOpType.bypass
                        ),
                    )
                    demote_pool_deps(gd)
                    swdge_dmas.append(gd)
                    swdge_names[gd.ins.name] = gd
                    return gd

                a_g = [gather(j, False) for j in range(4)]
                if i > 0:
                    # The DVE wait rides the tile-entry nop; drop it from the
                    # a-gathers (desc-gen follows the nop in program order).
                    for gd in a_g:
                        gd.ins.try_remove_dependency(i_idx.ins.name)
                        add_dep_helper(gd.ins, n.ins, False, "after idx8 nop")
                # One Pool nop PER CHAIN waits for that a-gather's DATA to
                # land (completion sems are per-DMA; engines pipeline
                # descriptors, so an earlier DMA's data is NOT implied by a
                # later one's completion).
                for j in range(4, TOPK):
                    cn = nc.gpsimd.nop()
                    add_dep_helper(cn.ins, a_g[j - 4].ins, True, "chain a done")
                    pool_insts[cn.ins.name] = cn
                    bd = gather(j, True)
                    bd.ins.try_remove_dependency(a_g[j - 4].ins.name)
                    add_dep_helper(bd.ins, cn.ins, False, "after chain nop")
                    # idx8 read happens at desc-gen, which is program-ordered
                    # behind this tile's DVE wait already.
                    bd.ins.try_remove_dependency(i_idx.ins.name)
                    add_dep_helper(bd.ins, i_idx.ins, False, "idx8 via entry wait")

                # Defer this tile's reduce+store until after the NEXT tile's
                # max/max_index: the DVE adds wait on the gather round trip,
                # and the DVE runs its program in order -- emitting them here
                # would stall the next tile's top-k behind the gathers.
                pending.append((i, Gq, swdge_dmas[-4:]))
                if i > 1:
                    emit_reduce_store(*pending.pop(0))
            # drain the deferred final tiles
            while pending:
                emit_reduce_store(*pending.pop(0))

            # Quiesce outstanding proc ticks with single-wait SP nops so the
            # kernel-tail drain finds its required ticks already observed
            # (the drain itself may carry only one sync wait).
            tail = xls + swdge_dmas[-9:] + [finals["tt"], finals["va"]] + i_idxs[-1:]
            for tgt in tail:
                n = nc.sync.nop()
                add_dep_helper(n.ins, tgt.ins, True, "tail quiesce")

    # Post-pass: walrus allows ONE sync wait per instruction. Drop waits
    # that are provably redundant: (a) a wait on the instruction's own
    # engine sem (program order), (b) an earlier SWDGE/DMASW lane tick when
    # a later lane tick is also waited (lanes are assigned round-robin in
    # ring order and each SDMA engine drains the ring FIFO, so the later
    # DMA's completion implies the earlier one's data landed).
    for f in nc.m.functions:
        for b in f.blocks:
            for ins in b.instructions:
                si = ins.sync_info
                if not si or len(si.on_wait) <= 1:
                    continue
                eng = str(getattr(ins, "engine", "")).split(".")[-1]
                own = f"{eng}_"
                keep = [w for w in si.on_wait if not w.ant_name.startswith(own)]
                if ins.name in tt3_names:
                    dve = [w for w in keep if w.ant_name.startswith("DVE_")]
                    if dve:
                        keep = dve
                sw = [w for w in keep if w.ant_name.startswith("DMASW")]
                if len(sw) > 1:
                    # ring ordinal: DMA n -> lane n%8, count (n//8+1)*16
                    def ordinal(w):
                        lane = int(w.ant_name[5:].split("_")[0])
                        return (w.wait_value // 16 - 1) * 8 + lane
                    best = max(sw, key=ordinal)
                    keep = [w for w in keep if not (w.ant_name.startswith("DMASW") and w is not best)]
                if len(keep) >= 1 and len(keep) < len(si.on_wait):
                    ins.sync_info = mybir.SyncInfo(
                        on_wait=keep, on_update=list(si.on_update)
                    )
    return nc


def _prep_wt(W: np.ndarray) -> np.ndarray:
    """W [e, q] f32 -> WT [q, e] bf16 contiguous."""
    return np.ascontiguousarray(W.T).astype(ml_dtypes.bfloat16)


_CACHED = {}


def _get_nc():
    if "nc" not in _CACHED:
        _CACHED["nc"] = build_bass()
    return _CACHED["nc"]


def kernel(x: np.ndarray, W: np.ndarray) -> np.ndarray:
    x = np.asarray(x, dtype=np.float32)
    W = np.asarray(W, dtype=np.float32)
    assert x.shape == (B, S, Q) and W.shape == (E, Q)

    nc = _get_nc()
    xf = x.reshape(T_TOTAL, Q)
    WT = _prep_wt(W)
    in_maps = [
        {
            "x": np.ascontiguousarray(xf[c * T_CORE : (c + 1) * T_CORE]),
            "wt": WT,
        }
        for c in range(N_CORES)
    ]
    res = run_bass_kernel_spmd(nc, in_maps, core_ids=list(range(N_CORES)))
    out = np.concatenate([r["out"] for r in res.results], axis=0)
    return np.ascontiguousarray(out.reshape(B, S, E).astype(np.float32))


# revision 60
# speedup vs baseline: 1.3671x; 1.0204x over previous
"""Trainium2 Bass kernel: NKQuantizer2 top-k masking (k=8).

reference:  kh = topk_hot(x, 8)          # [B,S,Q] 0/1 mask, top-8 per token
            out = einsum('bsq,eq->bse', kh, W)

Per token: out[t] = sum_{q in top8(x[t])} W[:, q] -- an 8-way embedding
gather-sum from W.T [Q, E].

Strategy (data-parallel over tokens across 8 cores; WT bf16 in HBM).
Per 128-token tile on each core:
  1. HWDGE DMA x tile [128, 8192] f32 HBM->SBUF (SP ring, 3-deep)
  2. DVE Max8 -> top-8 values; DVE MaxIndex -> their indices (exact,
     duplicate values resolved to successive positions, matching
     jax.lax.top_k first-occurrence tie-break)
  3. ONE SWDGE multi-index gather: G[p, j, :] = WT[idx8[p, j], :]
     (1024 descriptors of 1 KiB in a single instruction -- one Q7
     descriptor-gen pass, one DVE semaphore wait)
  4. GPSIMD tensor_tensor add tree over the j axis (3 levels, Pool
     engine -- DVE stays free): g8 = sum_j G[:, j, :]
  5. SWDGE store g8 -> out rows

Every DMA instruction may carry at most ONE semaphore wait (walrus
limit). Deps whose source is a Pool-engine instruction are demoted to
nosync ordering edges for SWDGE DMAs: their descriptor generation runs
on the Pool sequencer AFTER that instruction completed (program order),
so the semaphore would be redundant.
"""

import numpy as np
import ml_dtypes

import concourse.bass as bass
import concourse.mybir as mybir
import concourse.tile as tile
from concourse.bass_utils import run_bass_kernel_spmd
from concourse.tile_rust import add_dep_helper

B, S, Q, E, TOPK = 4, 2048, 8192, 512, 8
N_CORES = 8
P = 128
T_TOTAL = B * S                 # 8192 tokens
T_CORE = T_TOTAL // N_CORES     # 1024 tokens per core

F32 = mybir.dt.float32
BF16 = mybir.dt.bfloat16
U32 = mybir.dt.uint32


def build_bass(t_core=T_CORE, q=Q, e=E):
    """Build the per-core Bass program (SPMD: same program on all cores)."""
    n_tiles = t_core // P
    xbufs = min(3, n_tiles)

    nc = bass.Bass(trn_type="TRN2", target_bir_lowering=False)
    x_d = nc.dram_tensor("x", [t_core, q], F32, kind="ExternalInput")
    wt_d = nc.dram_tensor("wt", [q, e], BF16, kind="ExternalInput")
    out_d = nc.dram_tensor("out", [t_core, e], F32, kind="ExternalOutput")

    pool_insts = {}  # name -> Pool-engine compute instruction
    tt3_names = set()

    def demote_pool_deps(inst, keep=()):
        """Demote an instruction's sync deps on Pool COMPUTE instructions to
        nosync edges: the Pool engine executes serially, so anything issued
        later on Pool (compute or SWDGE descriptor-gen) starts only after
        those completed. Deps on DMA instructions (async data landing) are
        kept."""
        keep_names = {k.ins.name for k in keep}
        for name in list(inst.ins.sync_dependency_names()):
            if name in pool_insts and name not in keep_names:
                inst.ins.try_remove_dependency(name)
                add_dep_helper(inst.ins, pool_insts[name].ins, False, "Pool order")
        return inst

    with tile.TileContext(nc) as tc:
        with (
            tc.tile_pool(name="xpool", bufs=xbufs) as xpool,
            tc.tile_pool(name="spool", bufs=n_tiles) as spool,
            tc.tile_pool(name="ipool", bufs=n_tiles) as ipool,
            tc.tile_pool(name="Gpool", bufs=n_tiles) as Gpool,
            tc.tile_pool(name="gpool", bufs=n_tiles) as gpool,
        ):
            xts = [xpool.tile([P, q], F32, name="xt", tag="xt") for _ in range(xbufs)]
            i_idxs = []
            xls = []
            xl_names = set()
            swdge_dmas = []
            swdge_names = {}
            pending = []
            a_pending = []
            finals = {}
            dve_insts = {}
            dve_latest = [None]

            def reg_dve(op):
                dve_insts[op.ins.name] = op
                dve_latest[0] = op
                return op

            def emit_reduce_store(i, Gq, b_g):
                # Two in-place quarter adds on DVE (ordered after the newest
                # max_index via a nosync edge so they never block top-k),
                # final add on Pool, then the SWDGE store.
                # A nop ladder waits each quarter's own completion sem (one
                # wait per instruction), then one merged add folds quarters
                # 2,3 into 0,1.
                ladder = []
                for bg in b_g:
                    vn = reg_dve(nc.vector.nop())
                    add_dep_helper(vn.ins, bg.ins, True, "quarter done")
                    ladder.append(vn)
                vn1 = ladder[-1]
                va1 = reg_dve(nc.vector.tensor_tensor(
                    out=Gq[:, 0:2, :], in0=Gq[:, 0:2, :], in1=Gq[:, 2:4, :],
                    op=mybir.AluOpType.add,
                ))
                for vn in ladder:
                    # The add must sit AFTER the whole ladder in DVE program
                    # order -- without these edges the scheduler may hoist it
                    # above the waits.
                    add_dep_helper(va1.ins, vn.ins, False, "after ladder")
                    add_dep_helper(vn.ins, i_idxs[-1].ins, False, "after topk")
                for bg in b_g:
                    va1.ins.try_remove_dependency(bg.ins.name)
                    add_dep_helper(va1.ins, bg.ins, False, "via ring nop")
                # Hoist any remaining cross-engine sync deps (scheduler
                # artifacts on tail tiles) onto their own nops; demote
                # same-engine (DVE) deps to program order.
                for name in list(va1.ins.sync_dependency_names()):
                    if name in dve_insts:
                        va1.ins.try_remove_dependency(name)
                        add_dep_helper(va1.ins, dve_insts[name].ins, False, "DVE order")
                    elif name in swdge_names:
                        va1.ins.try_remove_dependency(name)
                        hn = reg_dve(nc.vector.nop())
                        add_dep_helper(hn.ins, swdge_names[name].ins, True, "hoist")
                        add_dep_helper(va1.ins, hn.ins, False, "after hoist nop")
                va2 = va1
                for v in (vn1, va1):
                    # Keep the whole reduce AFTER the newest max_index in DVE
                    # program order -- it waits on the gather round trip and
                    # must never stall the top-k stream.
                    add_dep_helper(v.ins, i_idxs[-1].ins, False, "after topk")
                tt3 = nc.gpsimd.tensor_tensor(
                    out=Gq[:, 0, :], in0=Gq[:, 0, :], in1=Gq[:, 1, :],
                    op=mybir.AluOpType.add,
                )
                pool_insts[tt3.ins.name] = tt3
                tt3_names.add(tt3.ins.name)
                st = nc.gpsimd.dma_start(
                    out_d[i * P : (i + 1) * P, :], Gq[:, 0, :]
                )
                demote_pool_deps(st)
                swdge_dmas.append(st)
                swdge_names[st.ins.name] = st
                finals["tt"] = tt3
                finals["va"] = va2
            for i in range(n_tiles):
                xt = xts[i % xbufs]
                # Load x in 4 column chunks: a single 4 MiB DMA queues 256
                # KiB per SDMA engine, which delays the gathers' completion
                # semaphores by up to ~9 us; 1 MiB chunks cap that at ~2.5 us.
                # All chunks ride the SP HWDGE ring in order, so chunk 3's
                # completion implies chunks 0-2 have landed on every engine.
                chunks = []
                spn = None
                if i >= xbufs:
                    # One SP nop carries every xt-slot WAR: the old slot
                    # readers are max/max_index of tile i-xbufs, covered by
                    # that tile's max_index tick. Tiles 0..xbufs-1 write
                    # fresh buffers -- any DVE dep on their chunks is a
                    # tracking artifact, dropped without replacement.
                    spn = nc.sync.nop()
                    add_dep_helper(spn.ins, i_idxs[i - xbufs].ins, True, "xt WAR")
                for c4 in range(4):
                    lo, hi = c4 * (q // 4), (c4 + 1) * (q // 4)
                    xc = nc.sync.dma_start(
                        xt[:, lo:hi], x_d[i * P : (i + 1) * P, lo:hi]
                    )
                    for name in list(xc.ins.sync_dependency_names()):
                        if name in dve_insts:
                            xc.ins.try_remove_dependency(name)
                        elif name in xl_names:
                            # old-chunk WAW, redundant under the WAR
                            xc.ins.try_remove_dependency(name)
                    if spn is not None:
                        add_dep_helper(xc.ins, spn.ins, False, "after WAR nop")
                    chunks.append(xc)
                xls.extend(chunks)
                for xc in chunks:
                    xl_names.add(xc.ins.name)

                s8 = spool.tile([P, 8], F32, name="s8", tag="s8")
                vm = reg_dve(nc.vector.max(out=s8[:], in_=xt[:]))
                # One wait: the last chunk's completion implies the rest.
                for xc in chunks[:-1]:
                    vm.ins.try_remove_dependency(xc.ins.name)
                    add_dep_helper(vm.ins, xc.ins, False, "ring order")
                idx8 = ipool.tile([P, 8], U32, name="idx8", tag="idx8")
                i_idx = reg_dve(nc.vector.max_index(
                    out=idx8[:], in_max=s8[:], in_values=xt[:]
                ))
                # max8 already waited on the last chunk; DVE order covers it.
                for xc in chunks:
                    i_idx.ins.try_remove_dependency(xc.ins.name)
                    add_dep_helper(i_idx.ins, xc.ins, False, "via max8 wait")
                i_idxs.append(i_idx)

                # 8 single-offset gathers (the HW-supported form) arranged as
                # 4 CCE-accumulate chains of depth 2 into quarters:
                #   Gq[:, j, :]  = WT[idx8[:, j]]        (bypass, bf16->f32)
                #   Gq[:, j, :] += WT[idx8[:, j+4]]      (CCE add)
                # Chain waits are hoisted onto Pool nops so each DMA carries
                # at most one (sem-lane) wait.
                Gq = Gpool.tile([P, 4, e], F32, name="Gq", tag="Gq")
                if i > 0:
                    # Hoist the idx8-ready (DVE) wait for this tile's gathers.
                    n = nc.gpsimd.nop()
                    add_dep_helper(n.ins, i_idx.ins, True, "idx8 ready")
                    pool_insts[n.ins.name] = n

                def gather(j, acc, _Gq=Gq, _idx8=idx8):
                    # bind THIS tile's tensors: the b-round invokes this
                    # closure one loop iteration later.
                    gd = nc.gpsimd.indirect_dma_start(
                        out=_Gq[:, j % 4, :],
                        out_offset=None,
                        in_=wt_d[:],
                        in_offset=bass.IndirectOffsetOnAxis(
                            ap=_idx8[:, j : j + 1], axis=0
                        ),
                        compute_op=(
                            mybir.AluOpType.add if acc else mybir.AluOpType.bypass
                        ),
                    )
                    demote_pool_deps(gd)
                    swdge_dmas.append(gd)
                    swdge_names[gd.ins.name] = gd
                    return gd

                a_g = [gather(j, False) for j in range(4)]
                if i > 0:
                    # The DVE wait rides the tile-entry nop; drop it from the
                    # a-gathers (desc-gen follows the nop in program order).
                    for gd in a_g:
                        gd.ins.try_remove_dependency(i_idx.ins.name)
                        add_dep_helper(gd.ins, n.ins, False, "after idx8 nop")
                def emit_b_round(bi, bGq, b_a_g, b_gather, b_iidx):
                    # One Pool nop PER CHAIN waits for that a-gather's DATA
                    # to land. Emitted one tile late, the a-completions have
                    # already landed and the nops pass without stalling.
                    b_g = []
                    for j in range(4, TOPK):
                        cn = nc.gpsimd.nop()
                        add_dep_helper(cn.ins, b_a_g[j - 4].ins, True, "chain a done")
                        pool_insts[cn.ins.name] = cn
                        bd = b_gather(j, True)
                        bd.ins.try_remove_dependency(b_a_g[j - 4].ins.name)
                        add_dep_helper(bd.ins, cn.ins, False, "after chain nop")
                        bd.ins.try_remove_dependency(b_iidx.ins.name)
                        add_dep_helper(bd.ins, b_iidx.ins, False, "idx via a-round")
                        b_g.append(bd)
                    pending.append((bi, bGq, b_g))
                    if len(pending) > 1:
                        emit_reduce_store(*pending.pop(0))

                # Software-pipeline the CCE chains across tiles: tile i-1's
                # b-round (CCE adds) is emitted after tile i's a-round.
                a_pending.append((i, Gq, a_g, gather, i_idx))
                if i > 0:
                    emit_b_round(*a_pending.pop(0))
            # drain the deferred final rounds
            while a_pending:
                emit_b_round(*a_pending.pop(0))
            while pending:
                emit_reduce_store(*pending.pop(0))

            # Quiesce outstanding proc ticks with single-wait SP nops so the
            # kernel-tail drain finds its required ticks already observed
            # (the drain itself may carry only one sync wait).
            tail = xls + swdge_dmas[-9:] + [finals["tt"], finals["va"]] + i_idxs[-1:]
            for tgt in tail:
                n = nc.sync.nop()
                add_dep_helper(n.ins, tgt.ins, True, "tail quiesce")

    # Post-pass: walrus allows ONE sync wait per instruction. Drop waits
    # that are provably redundant: (a) a wait on the instruction's own
    # engine sem (program order), (b) an earlier SWDGE/DMASW lane tick when
    # a later lane tick is also waited (lanes are assigned round-robin in
    # ring order and each SDMA engine drains the ring FIFO, so the later
    # DMA's completion implies the earlier one's data landed).
    for f in nc.m.functions:
        for b in f.blocks:
            for ins in b.instructions:
                si = ins.sync_info
                if not si or len(si.on_wait) <= 1:
                    continue
                eng = str(getattr(ins, "engine", "")).split(".")[-1]
                own = f"{eng}_"
                keep = [w for w in si.on_wait if not w.ant_name.startswith(own)]
                if ins.name in tt3_names:
                    dve = [w for w in keep if w.ant_name.startswith("DVE_")]
                    if dve:
                        keep = dve
                sw = [w for w in keep if w.ant_name.startswith("DMASW")]
                if len(sw) > 1:
                    # ring ordinal: DMA n -> lane n%8, count (n//8+1)*16
                    def ordinal(w):
                        lane = int(w.ant_name[5:].split("_")[0])
                        return (w.wait_value // 16 - 1) * 8 + lane
                    best = max(sw, key=ordinal)
                    keep = [w for w in keep if not (w.ant_name.startswith("DMASW") and w is not best)]
                if len(keep) >= 1 and len(keep) < len(si.on_wait):
                    ins.sync_info = mybir.SyncInfo(
                        on_wait=keep, on_update=list(si.on_update)
                    )
    return nc


def _prep_wt(W: np.ndarray) -> np.ndarray:
    """W [e, q] f32 -> WT [q, e] bf16 contiguous."""
    return np.ascontiguousarray(W.T).astype(ml_dtypes.bfloat16)


_CACHED = {}


def _get_nc():
    if "nc" not in _CACHED:
        _CACHED["nc"] = build_bass()
    return _CACHED["nc"]


def kernel(x: np.ndarray, W: np.ndarray) -> np.ndarray:
    x = np.asarray(x, dtype=np.float32)
    W = np.asarray(W, dtype=np.float32)
    assert x.shape == (B, S, Q) and W.shape == (E, Q)

    nc = _get_nc()
    xf = x.reshape(T_TOTAL, Q)
    WT = _prep_wt(W)
    in_maps = [
        {
            "x": np.ascontiguousarray(xf[c * T_CORE : (c + 1) * T_CORE]),
            "wt": WT,
        }
        for c in range(N_CORES)
    ]
    res = run_bass_kernel_spmd(nc, in_maps, core_ids=list(range(N_CORES)))
    out = np.concatenate([r["out"] for r in res.results], axis=0)
    return np.ascontiguousarray(out.reshape(B, S, E).astype(np.float32))


# revision 61
# speedup vs baseline: 1.4955x; 1.0939x over previous
"""Trainium2 Bass kernel: NKQuantizer2 top-k masking (k=8).

reference:  kh = topk_hot(x, 8)          # [B,S,Q] 0/1 mask, top-8 per token
            out = einsum('bsq,eq->bse', kh, W)

Per token: out[t] = sum_{q in top8(x[t])} W[:, q] -- an 8-way embedding
gather-sum from W.T [Q, E].

Strategy (data-parallel over tokens across 8 cores; WT bf16 in HBM).
Per 128-token tile on each core:
  1. HWDGE DMA x tile [128, 8192] f32 HBM->SBUF (SP ring, 3-deep)
  2. DVE Max8 -> top-8 values; DVE MaxIndex -> their indices (exact,
     duplicate values resolved to successive positions, matching
     jax.lax.top_k first-occurrence tie-break)
  3. ONE SWDGE multi-index gather: G[p, j, :] = WT[idx8[p, j], :]
     (1024 descriptors of 1 KiB in a single instruction -- one Q7
     descriptor-gen pass, one DVE semaphore wait)
  4. GPSIMD tensor_tensor add tree over the j axis (3 levels, Pool
     engine -- DVE stays free): g8 = sum_j G[:, j, :]
  5. SWDGE store g8 -> out rows

Every DMA instruction may carry at most ONE semaphore wait (walrus
limit). Deps whose source is a Pool-engine instruction are demoted to
nosync ordering edges for SWDGE DMAs: their descriptor generation runs
on the Pool sequencer AFTER that instruction completed (program order),
so the semaphore would be redundant.
"""

import numpy as np
import ml_dtypes

import concourse.bass as bass
import concourse.mybir as mybir
import concourse.tile as tile
from concourse.bass_utils import run_bass_kernel_spmd
from concourse.tile_rust import add_dep_helper

B, S, Q, E, TOPK = 4, 2048, 8192, 512, 8
N_CORES = 8
P = 128
T_TOTAL = B * S                 # 8192 tokens
T_CORE = T_TOTAL // N_CORES     # 1024 tokens per core

F32 = mybir.dt.float32
BF16 = mybir.dt.bfloat16
U32 = mybir.dt.uint32


def build_bass(t_core=T_CORE, q=Q, e=E):
    """Build the per-core Bass program (SPMD: same program on all cores)."""
    n_tiles = t_core // P
    xbufs = min(3, n_tiles)

    nc = bass.Bass(trn_type="TRN2", target_bir_lowering=False)
    x_d = nc.dram_tensor("x", [t_core, q], F32, kind="ExternalInput")
    wt_d = nc.dram_tensor("wt", [q, e], BF16, kind="ExternalInput")
    out_d = nc.dram_tensor("out", [t_core, e], F32, kind="ExternalOutput")

    pool_insts = {}  # name -> Pool-engine compute instruction
    tt3_names = set()

    def demote_pool_deps(inst, keep=()):
        """Demote an instruction's sync deps on Pool COMPUTE instructions to
        nosync edges: the Pool engine executes serially, so anything issued
        later on Pool (compute or SWDGE descriptor-gen) starts only after
        those completed. Deps on DMA instructions (async data landing) are
        kept."""
        keep_names = {k.ins.name for k in keep}
        for name in list(inst.ins.sync_dependency_names()):
            if name in pool_insts and name not in keep_names:
                inst.ins.try_remove_dependency(name)
                add_dep_helper(inst.ins, pool_insts[name].ins, False, "Pool order")
        return inst

    with tile.TileContext(nc) as tc:
        with (
            tc.tile_pool(name="xpool", bufs=xbufs) as xpool,
            tc.tile_pool(name="spool", bufs=n_tiles) as spool,
            tc.tile_pool(name="ipool", bufs=n_tiles) as ipool,
            tc.tile_pool(name="Gpool", bufs=n_tiles) as Gpool,
            tc.tile_pool(name="gpool", bufs=n_tiles) as gpool,
        ):
            xts = [xpool.tile([P, q], F32, name="xt", tag="xt") for _ in range(xbufs)]
            i_idxs = []
            xls = []
            xl_names = set()
            swdge_dmas = []
            swdge_names = {}
            pending = []
            a_pending = []
            finals = {}
            dve_insts = {}
            dve_latest = [None]

            def reg_dve(op):
                dve_insts[op.ins.name] = op
                dve_latest[0] = op
                return op

            def emit_reduce_store(i, Gq, b_g):
                # Two in-place quarter adds on DVE (ordered after the newest
                # max_index via a nosync edge so they never block top-k),
                # final add on Pool, then the SWDGE store.
                # A nop ladder waits each quarter's own completion sem (one
                # wait per instruction), then one merged add folds quarters
                # 2,3 into 0,1.
                ladder = []
                for bg in b_g:
                    vn = reg_dve(nc.vector.nop())
                    add_dep_helper(vn.ins, bg.ins, True, "slice done")
                    ladder.append(vn)
                vn1 = ladder[-1]
                va0 = reg_dve(nc.vector.tensor_tensor(
                    out=Gq[:, 0:4, :], in0=Gq[:, 0:4, :], in1=Gq[:, 4:8, :],
                    op=mybir.AluOpType.add,
                ))
                for vn in ladder:
                    add_dep_helper(va0.ins, vn.ins, False, "after ladder")
                for bg in b_g:
                    va0.ins.try_remove_dependency(bg.ins.name)
                    add_dep_helper(va0.ins, bg.ins, False, "via ladder")
                va1 = reg_dve(nc.vector.tensor_tensor(
                    out=Gq[:, 0:2, :], in0=Gq[:, 0:2, :], in1=Gq[:, 2:4, :],
                    op=mybir.AluOpType.add,
                ))
                for vn in ladder:
                    # The add must sit AFTER the whole ladder in DVE program
                    # order -- without these edges the scheduler may hoist it
                    # above the waits.
                    add_dep_helper(va1.ins, vn.ins, False, "after ladder")
                    add_dep_helper(vn.ins, i_idxs[-1].ins, False, "after topk")
                for bg in b_g:
                    va1.ins.try_remove_dependency(bg.ins.name)
                    add_dep_helper(va1.ins, bg.ins, False, "via ring nop")
                # Hoist any remaining cross-engine sync deps (scheduler
                # artifacts on tail tiles) onto their own nops; demote
                # same-engine (DVE) deps to program order.
                for name in list(va1.ins.sync_dependency_names()):
                    if name in dve_insts:
                        va1.ins.try_remove_dependency(name)
                        add_dep_helper(va1.ins, dve_insts[name].ins, False, "DVE order")
                    elif name in swdge_names:
                        va1.ins.try_remove_dependency(name)
                        hn = reg_dve(nc.vector.nop())
                        add_dep_helper(hn.ins, swdge_names[name].ins, True, "hoist")
                        add_dep_helper(va1.ins, hn.ins, False, "after hoist nop")
                va2 = va1
                for v in (vn1, va1):
                    # Keep the whole reduce AFTER the newest max_index in DVE
                    # program order -- it waits on the gather round trip and
                    # must never stall the top-k stream.
                    add_dep_helper(v.ins, i_idxs[-1].ins, False, "after topk")
                g8 = gpool.tile([P, e], F32, name="g8", tag="g8")
                tt3 = nc.gpsimd.tensor_tensor(
                    out=g8[:], in0=Gq[:, 0, :], in1=Gq[:, 1, :],
                    op=mybir.AluOpType.add,
                )
                pool_insts[tt3.ins.name] = tt3
                tt3_names.add(tt3.ins.name)
                st = nc.gpsimd.dma_start(
                    out_d[i * P : (i + 1) * P, :], g8[:]
                )
                demote_pool_deps(st)
                swdge_dmas.append(st)
                swdge_names[st.ins.name] = st
                finals["tt"] = tt3
                finals["va"] = va2
            for i in range(n_tiles):
                xt = xts[i % xbufs]
                # Load x in 4 column chunks: a single 4 MiB DMA queues 256
                # KiB per SDMA engine, which delays the gathers' completion
                # semaphores by up to ~9 us; 1 MiB chunks cap that at ~2.5 us.
                # All chunks ride the SP HWDGE ring in order, so chunk 3's
                # completion implies chunks 0-2 have landed on every engine.
                chunks = []
                spn = None
                if i >= xbufs:
                    # One SP nop carries every xt-slot WAR: the old slot
                    # readers are max/max_index of tile i-xbufs, covered by
                    # that tile's max_index tick. Tiles 0..xbufs-1 write
                    # fresh buffers -- any DVE dep on their chunks is a
                    # tracking artifact, dropped without replacement.
                    spn = nc.sync.nop()
                    add_dep_helper(spn.ins, i_idxs[i - xbufs].ins, True, "xt WAR")
                for c4 in range(4):
                    lo, hi = c4 * (q // 4), (c4 + 1) * (q // 4)
                    xc = nc.sync.dma_start(
                        xt[:, lo:hi], x_d[i * P : (i + 1) * P, lo:hi]
                    )
                    for name in list(xc.ins.sync_dependency_names()):
                        if name in dve_insts:
                            xc.ins.try_remove_dependency(name)
                        elif name in xl_names:
                            # old-chunk WAW, redundant under the WAR
                            xc.ins.try_remove_dependency(name)
                    if spn is not None:
                        add_dep_helper(xc.ins, spn.ins, False, "after WAR nop")
                    chunks.append(xc)
                xls.extend(chunks)
                for xc in chunks:
                    xl_names.add(xc.ins.name)

                s8 = spool.tile([P, 8], F32, name="s8", tag="s8")
                vm = reg_dve(nc.vector.max(out=s8[:], in_=xt[:]))
                # One wait: the last chunk's completion implies the rest.
                for xc in chunks[:-1]:
                    vm.ins.try_remove_dependency(xc.ins.name)
                    add_dep_helper(vm.ins, xc.ins, False, "ring order")
                idx8 = ipool.tile([P, 8], U32, name="idx8", tag="idx8")
                i_idx = reg_dve(nc.vector.max_index(
                    out=idx8[:], in_max=s8[:], in_values=xt[:]
                ))
                # max8 already waited on the last chunk; DVE order covers it.
                for xc in chunks:
                    i_idx.ins.try_remove_dependency(xc.ins.name)
                    add_dep_helper(i_idx.ins, xc.ins, False, "via max8 wait")
                i_idxs.append(i_idx)

                # 8 single-offset gathers (the HW-supported form) arranged as
                # 4 CCE-accumulate chains of depth 2 into quarters:
                #   Gq[:, j, :]  = WT[idx8[:, j]]        (bypass, bf16->f32)
                #   Gq[:, j, :] += WT[idx8[:, j+4]]      (CCE add)
                # Chain waits are hoisted onto Pool nops so each DMA carries
                # at most one (sem-lane) wait.
                Gq = Gpool.tile([P, TOPK, e], BF16, name="Gq", tag="Gq")
                if i > 0:
                    # Hoist the idx8-ready (DVE) wait for this tile's gathers.
                    n = nc.gpsimd.nop()
                    add_dep_helper(n.ins, i_idx.ins, True, "idx8 ready")
                    pool_insts[n.ins.name] = n

                def gather(j, acc, _Gq=Gq, _idx8=idx8):
                    gd = nc.gpsimd.indirect_dma_start(
                        out=_Gq[:, j, :],
                        out_offset=None,
                        in_=wt_d[:],
                        in_offset=bass.IndirectOffsetOnAxis(
                            ap=_idx8[:, j : j + 1], axis=0
                        ),
                        compute_op=mybir.AluOpType.bypass,
                    )
                    demote_pool_deps(gd)
                    swdge_dmas.append(gd)
                    swdge_names[gd.ins.name] = gd
                    return gd

                a_g = [gather(j, False) for j in range(TOPK)]
                if i > 0:
                    # The DVE wait rides the tile-entry nop; drop it from the
                    # gathers (desc-gen follows the nop in program order).
                    for gd in a_g:
                        gd.ins.try_remove_dependency(i_idx.ins.name)
                        add_dep_helper(gd.ins, n.ins, False, "after idx8 nop")

                # Defer this tile's reduce+store until after the NEXT tile's
                # max/max_index so the gather round trip never stalls top-k.
                pending.append((i, Gq, a_g))
                if i > 1:
                    emit_reduce_store(*pending.pop(0))
            # drain the deferred final tiles
            while pending:
                emit_reduce_store(*pending.pop(0))

            # Quiesce outstanding proc ticks with single-wait SP nops so the
            # kernel-tail drain finds its required ticks already observed
            # (the drain itself may carry only one sync wait).
            tail = xls + swdge_dmas[-9:] + [finals["tt"], finals["va"]] + i_idxs[-1:]
            for tgt in tail:
                n = nc.sync.nop()
                add_dep_helper(n.ins, tgt.ins, True, "tail quiesce")

    # Post-pass: walrus allows ONE sync wait per instruction. Drop waits
    # that are provably redundant: (a) a wait on the instruction's own
    # engine sem (program order), (b) an earlier SWDGE/DMASW lane tick when
    # a later lane tick is also waited (lanes are assigned round-robin in
    # ring order and each SDMA engine drains the ring FIFO, so the later
    # DMA's completion implies the earlier one's data landed).
    for f in nc.m.functions:
        for b in f.blocks:
            for ins in b.instructions:
                si = ins.sync_info
                if not si or len(si.on_wait) <= 1:
                    continue
                eng = str(getattr(ins, "engine", "")).split(".")[-1]
                own = f"{eng}_"
                keep = [w for w in si.on_wait if not w.ant_name.startswith(own)]
                if ins.name in tt3_names:
                    dve = [w for w in keep if w.ant_name.startswith("DVE_")]
                    if dve:
                        keep = dve
                sw = [w for w in keep if w.ant_name.startswith("DMASW")]
                if len(sw) > 1:
                    # ring ordinal: DMA n -> lane n%8, count (n//8+1)*16
                    def ordinal(w):
                        lane = int(w.ant_name[5:].split("_")[0])
                        return (w.wait_value // 16 - 1) * 8 + lane
                    best = max(sw, key=ordinal)
                    keep = [w for w in keep if not (w.ant_name.startswith("DMASW") and w is not best)]
                if len(keep) >= 1 and len(keep) < len(si.on_wait):
                    ins.sync_info = mybir.SyncInfo(
                        on_wait=keep, on_update=list(si.on_update)
                    )
    return nc


def _prep_wt(W: np.ndarray) -> np.ndarray:
    """W [e, q] f32 -> WT [q, e] bf16 contiguous."""
    return np.ascontiguousarray(W.T).astype(ml_dtypes.bfloat16)


_CACHED = {}


def _get_nc():
    if "nc" not in _CACHED:
        _CACHED["nc"] = build_bass()
    return _CACHED["nc"]


def kernel(x: np.ndarray, W: np.ndarray) -> np.ndarray:
    x = np.asarray(x, dtype=np.float32)
    W = np.asarray(W, dtype=np.float32)
    assert x.shape == (B, S, Q) and W.shape == (E, Q)

    nc = _get_nc()
    xf = x.reshape(T_TOTAL, Q)
    WT = _prep_wt(W)
    in_maps = [
        {
            "x": np.ascontiguousarray(xf[c * T_CORE : (c + 1) * T_CORE]),
            "wt": WT,
        }
        for c in range(N_CORES)
    ]
    res = run_bass_kernel_spmd(nc, in_maps, core_ids=list(range(N_CORES)))
    out = np.concatenate([r["out"] for r in res.results], axis=0)
    return np.ascontiguousarray(out.reshape(B, S, E).astype(np.float32))
